# revision 41
# baseline (speedup 1.0000x reference)
"""Multi-head attention forward (B=8, S=1024, H=16, D=64) on 8 TRN2 NeuronCores.

Sharding: pure data-parallel over batch — core b computes batch element b
end-to-end (QKV projections + 16-head attention). Zero collectives.

Host-side marshalling (inside kernel(), before device launch): x_from/x_to
are transposed and weights cast to bf16, both pre-tiled into the exact
SBUF-partition-major images the kernel loads, so every DMA is 128 fat
contiguous descriptors. This halves HBM reads (10MB/core) and removes all
on-device transposes and casts; numerics match casting on-chip.

Per-core dataflow (bf16 matmuls, fp32 PSUM accumulation):
  - Q/K projections in [dims, s] orientation (lhsT = W block, rhs = x^T);
    bias added on the psum->sbuf move (per-partition tensor_scalar).
  - scores split per (key-block, q-half): the two heads' K=64 matmuls sit
    adjacent in the PE stream on disjoint row-groups, running concurrently;
    the [128,1024] scores psum tiles rotate 3-deep so scores for slot k
    wait only on exp(k-3) — the scores<->exp serial chain is broken and
    ScalarE runs its 16 exps/pair nearly back-to-back.
  - V projection in [keys, dims] orientation (lhsT = x_to^T block,
    rhs = Wv block) lands V'|ones strips directly — no PE transposes.
    V bias is a PE outer-product broadcast added on the psum->sbuf move.
  - ctx re-oriented: stationary = Et [keys, q-block] (full 128x128 array,
    FWL-loaded under the 65-cycle stream), moving = V'|ones [keys, 65].
    ~35ns per matmul; the softmax denominator lands in psum column 64 —
    no output transpose; normalize = reciprocal + tensor_scalar_mul.
  - steady pairs run 16 exp-cadence slots; fillers are whole chains
    (PSUM accumulation groups must never interleave in the PE stream:
    a start=True matmul from any other group clears has_written state
    bank-wide and drops the open group's partial sum). The slot order is
    hill-climbed offline (sim_sched.py) to keep the PE >99% busy while
    the exp chain stays saturated: back(p-1) ctx, V-proj(p), Q/K(p+1),
    and pair p+2's first Q-half as the boundary filler.
  - ramp: the first-scores-critical 3MB is split across three DMA queues
    (sync/scalar/gpsimd) in consumption order; the remaining ~7MB issues
    ungated behind it. A short PE warm-up keeps HAM from starting the
    first chains at half clock. The drain splits the last output store
    so stores overlap the final ctx groups.
"""

import numpy as np
import ml_dtypes
from contextlib import ExitStack

import concourse.bass as bass
import concourse.mybir as mybir
import concourse.tile as tile
from concourse import bacc
from concourse.masks import make_identity
from concourse.bass_utils import run_bass_kernel_spmd

B, S, H, D = 8, 1024, 16, 64
W = H * D  # 1024
P = 128
N_CORES = 8
F32 = mybir.dt.float32
BF16 = mybir.dt.bfloat16
AF = mybir.ActivationFunctionType

ST = S // P   # 8 s-tiles (also key-blocks jt / q-tiles qt)
KT_ = W // P  # 8 contraction tiles
NP = H // 2   # 8 head pairs
HD1 = D + 1   # 65: V' width per head (V cols + ones col)
VW = 2 * HD1  # 130: V' slot width per key-block (two heads)
FINE = 2 * P  # first weight columns loaded as one early strided DMA


def build_kernel():
    nc = bacc.Bacc(trn_type="TRN2", target_bir_lowering=False, debug=False,
                   num_devices=N_CORES)

    # inputs arrive pre-tiled to SBUF-partition-major layouts (see run()):
    #   x: [128, (ih, g, k2, s)]  w fine: [128, (kt, 256)]  bulk: [128,(kt,768)]
    # so every load is 128 fat contiguous descriptors.
    xfT_ext = nc.dram_tensor("xfT", [P, KT_ * S], BF16, kind="ExternalInput").ap()
    xtT_ext = nc.dram_tensor("xtT", [P, KT_ * S], BF16, kind="ExternalInput").ap()
    wf_exts, wb_exts = {}, {}
    for nm, wn in (("q", "Wq"), ("k", "Wk"), ("v", "Wv")):
        wf_exts[nm] = nc.dram_tensor(
            wn + "f", [P, KT_ * FINE], BF16, kind="ExternalInput").ap()
        wb_exts[nm] = nc.dram_tensor(
            wn + "b", [P, KT_ * (W - FINE)], BF16, kind="ExternalInput").ap()
    bq_ext = nc.dram_tensor("bq", [W], F32, kind="ExternalInput").ap()
    bk_ext = nc.dram_tensor("bk", [W], F32, kind="ExternalInput").ap()
    bv_ext = nc.dram_tensor("bv", [W], F32, kind="ExternalInput").ap()
    out_ext = nc.dram_tensor("out", [S, W], F32, kind="ExternalOutput").ap()
    x_exts = {"f": xfT_ext, "t": xtT_ext}

    with tile.TileContext(nc) as tc, ExitStack() as top:
        const = top.enter_context(tc.tile_pool(name="const", bufs=1))
        big = top.enter_context(tc.tile_pool(name="big", bufs=1))
        pp_pool = top.enter_context(tc.tile_pool(name="pp", bufs=1))
        et_pool = top.enter_context(tc.tile_pool(name="et", bufs=16))
        sm_pool = top.enter_context(tc.tile_pool(name="sm", bufs=1))
        # PSUM: scores 3 x [128,1024] rotating (6 banks) + one shared
        # 2-bank pool for proj/vproj/ctx chains = 8
        ps_s = top.enter_context(
            tc.tile_pool(name="ps_s", bufs=3, space="PSUM"))
        ps_w = top.enter_context(
            tc.tile_pool(name="ps_w", bufs=2, space="PSUM"))
        ps_c = ps_w

        idf32 = const.tile([8, 8], F32, tag="idf32")
        make_identity(nc, idf32[:])
        ones1 = const.tile([1, P], F32, tag="ones1")
        nc.vector.memset(ones1[:], 1.0)
        # explicit zero bias for the exp activations: the float-bias path
        # materializes a const tensor via the (slow) const DMA queue, which
        # stalled the first exp ~19us
        zbias = const.tile([P, 1], F32, tag="zbias")
        nc.vector.memset(zbias[:], 0.0)

        # ---- SBUF weight / x tiles (free layouts match the pre-tiled
        # DRAM images 1:1, so DMAs are fully contiguous) ----
        wfsb = {nm: big.tile([P, KT_ * FINE], BF16, tag=f"wf{nm}",
                             name=f"wf{nm}") for nm in ("q", "k", "v")}
        wbsb = {nm: big.tile([P, KT_ * (W - FINE)], BF16, tag=f"wb{nm}",
                             name=f"wb{nm}") for nm in ("q", "k", "v")}
        xsb = {h: big.tile([P, KT_ * S], BF16, tag=f"x{h}", name=f"x{h}")
               for h in ("f", "t")}

        def w_slice(nm, mt, kt):
            if mt < 2:
                o = kt * FINE + mt * P
                return wfsb[nm][:, o: o + P]
            o = kt * (W - FINE) + (mt - 2) * P
            return wbsb[nm][:, o: o + P]

        def x_slice(h, kt, s0, sn):
            # free layout: (ih, g, k2, s) blocks of 512
            ih, s_off = s0 // 512, s0 % 512
            g, k2 = kt // 4, kt % 4
            o = ih * 4096 + g * 2048 + k2 * 512 + s_off
            return xsb[h][:, o: o + sn]

        # ---- DMA issue ----
        # Only SP (sync) and GpSimd issue DMAs: a dma_start on a busy HWDGE
        # ring BLOCKS the issuing engine, so the Scalar engine (exp chain!)
        # must never carry transfers.
        #   sync  (HWDGE): xf h0, wq fine, xf h1, wq bulk + output stores
        #   gpsimd(SWDGE): biases, xt h0, wk fine, xt h1, wv fine, wk/wv bulk
        brow = const.tile([8, 3 * P], F32, tag="brow")
        brow_v = const.tile([1, W], F32, tag="brow_v")

        def x_half_load(h, ih, g, eng):
            o = ih * 4096 + g * 2048
            eng.dma_start(xsb[h][:, o: o + 2048],
                          x_exts[h][:, o: o + 2048])

        def w_fine_load(nm, g, eng):
            o = g * 4 * FINE
            eng.dma_start(wfsb[nm][:, o: o + 4 * FINE],
                          wf_exts[nm][:, o: o + 4 * FINE])

        def w_bulk_load(nm, kt, eng):
            o = kt * (W - FINE)
            eng.dma_start(wbsb[nm][:, o: o + (W - FINE)],
                          wb_exts[nm][:, o: o + (W - FINE)])

        # critical set split across all three queues (~1.5MB each):
        # sync: xf h0 + wq fine; scalar: xt h0 + wk fine (few fat issues,
        # done by ~8us, long before the first exp); gpsimd: biases + wv fine
        x_half_load("f", 0, 0, nc.sync)
        w_fine_load("q", 0, nc.sync)
        x_half_load("f", 0, 1, nc.sync)
        w_fine_load("q", 1, nc.sync)
        for kt in range(KT_):
            w_bulk_load("q", kt, nc.sync)
        x_half_load("t", 0, 0, nc.scalar)
        w_fine_load("k", 0, nc.scalar)
        x_half_load("t", 0, 1, nc.scalar)
        w_fine_load("k", 1, nc.scalar)
        for i, b_ext in enumerate((bq_ext, bk_ext, bv_ext)):
            nc.gpsimd.dma_start(brow[:, i * P:(i + 1) * P],
                                b_ext.rearrange("(t p) -> t p", p=P))
        nc.gpsimd.dma_start(brow_v[:], bv_ext.rearrange("(a w) -> a w", a=1))
        w_fine_load("v", 0, nc.gpsimd)
        w_fine_load("v", 1, nc.gpsimd)
        gate = const.tile([1, 1], BF16, tag="gate")

        def issue_gated_loads(qt0):
            # gpsimd-stream gate: wait for the first Q projection before
            # issuing the remaining ~7MB so the flood can't starve the
            # ramp-critical sync-queue transfers
            _ = qt0  # ungated: with fat descriptors the flood no longer clogs
            x_half_load("f", 1, 0, nc.gpsimd)
            x_half_load("t", 1, 0, nc.gpsimd)
            x_half_load("f", 1, 1, nc.gpsimd)
            x_half_load("t", 1, 1, nc.gpsimd)
            for nm in ("k", "v"):
                for kt in range(KT_):
                    w_bulk_load(nm, kt, nc.gpsimd)

        # ---- PE warm-up pads: dummy matmuls keep HAM un-throttled
        # (2.4 GHz) across the DMA-bound ramp; each pad block gets its own
        # psum tile so the ps_w rotation stays emission-ordered ----
        onesb = const.tile([1, 512], BF16, tag="onesb")
        nc.vector.memset(onesb[:], 1.0)

        def pe_pad(n):
            wps = ps_w.tile([P, 512], F32, tag="psw", name="wps")
            for _ in range(n):
                nc.tensor.matmul(wps[:], lhsT=onesb[:, 0:P], rhs=onesb[:],
                                 start=True, stop=True)

        pe_pad(14)


        # ---- bias transpose: [8,128] rows -> [128,8] columns ----
        b_sb = const.tile([P, 24], F32, tag="b_sb")
        bps = ps_w.tile([P, 24], F32, tag="psw", name="bps")
        for i in range(3):
            nc.tensor.transpose(bps[:, i * 8:(i + 1) * 8],
                                brow[:, i * P:(i + 1) * P], idf32[:])
        nc.vector.tensor_copy(b_sb[:], bps[:])

        # ---- building blocks ----
        # NOTE: every PSUM accumulation group (proj / vproj / ctx chain)
        # must be emitted as an uninterrupted run of PE matmuls: a
        # start=True matmul from ANY other group — even one targeting a
        # different PSUM bank — clears has_written state and silently
        # drops the open group's partial sum.
        def proj_half(dst, nm, mt, ih):
            # Q/K proj half [dims 128, s 512]
            ps = ps_w.tile([P, 512], F32, tag="psw", name="pp")
            for kt in range(KT_):
                nc.tensor.matmul(
                    ps[:],
                    lhsT=w_slice(nm, mt, kt),
                    rhs=x_slice("f" if nm == "q" else "t", kt, ih * 512, 512),
                    start=(kt == 0), stop=(kt == KT_ - 1))
            bof = {"q": 0, "k": 8}[nm]
            nc.vector.tensor_scalar_add(
                dst[:, ih * 512:(ih + 1) * 512], ps[:],
                b_sb[:, bof + mt:bof + mt + 1])

        def scores_ih(QT, KT2, jt, ih, Et):
            # one q-half of one key-block: the two heads' K=64 matmuls sit
            # adjacent in the PE stream on disjoint row-groups (0-63 /
            # 64-127), running concurrently. The [128,1024] psum tiles
            # rotate 3-deep so scores for slot k only wait on exp(k-3) --
            # the scores<->exp serial chain of a single big tile is gone.
            pss = ps_s.tile([P, 1024], F32, tag="pss", name="pss")
            for hh in range(2):
                ho = hh * D
                nc.tensor.matmul(
                    pss[:, hh * 512:(hh + 1) * 512],
                    lhsT=KT2[ho:ho + D, jt * P:(jt + 1) * P],
                    rhs=QT[ho:ho + D, ih * 512:(ih + 1) * 512],
                    start=True, stop=True)
            if ih == 0:
                Et[jt] = et_pool.tile([P, 2 * S], BF16, tag="et", name="et")
            # Et layout: [keys, (ih, hh, q512)]
            nc.scalar.activation(Et[jt][:, ih * 1024:(ih + 1) * 1024],
                                 pss[:], AF.Exp, bias=zbias[:], scale=0.125)

        # bb_all[p, c] = bv[c] for all partitions p (PE outer products);
        # emitted after the first scores slot so the cold fp32 outer
        # products don't block the ramp-critical Q/K chains
        bb_all = const.tile([P, W], BF16, tag="bb_all")

        def build_bb():
            for half in range(2):
                pb = ps_w.tile([P, 512], F32, tag="psw", name="pb")
                nc.tensor.matmul(pb[:], lhsT=ones1[:],
                                 rhs=brow_v[:, half * 512:(half + 1) * 512],
                                 start=True, stop=True)
                nc.vector.tensor_copy(bb_all[:, half * 512:(half + 1) * 512],
                                      pb[:])


        def vproj_jt(cur, jt):
            # V proj for key-block jt: out [keys 128, dims 128]
            mt = cur["mt"]
            pv = ps_c.tile([P, P], F32, tag="psw", name="pv")
            for kt in range(KT_):
                nc.tensor.matmul(
                    pv[:],
                    lhsT=x_slice("t", kt, jt * P, P),
                    rhs=w_slice("v", mt, kt),
                    start=(kt == 0), stop=(kt == KT_ - 1))
            # add bias broadcast, write V' slot (cols 0:64 / 65:129; 64,129=ones)
            nc.vector.tensor_tensor(
                cur["Vp"].rearrange("p (j g c) -> p j g c", g=2, c=HD1)[
                    :, jt, :, 0:D],
                pv.rearrange("p (g c) -> p g c", c=D),
                bb_all[:, mt * P:(mt + 1) * P].rearrange(
                    "p (g c) -> p g c", c=D),
                mybir.AluOpType.add)

        def vprime_ones(Vp):
            nc.vector.memset(
                Vp.rearrange("p (j g c) -> p j g c", g=2, c=HD1)[
                    :, :, :, D:HD1], 1.0)

        def ctx_qt(prev, qt):
            # ctx for q-tile qt, both heads: stationary = Et [keys, q 128]
            # (full array), moving = V'|ones [keys, 65]; denominator lands
            # in cols 64 / 129.
            Vp, Et = prev["Vp"], prev["Et"]
            # per-head psum slot stride 66: keeps the two accumulation
            # groups in disjoint 8-byte PSUM units (65 would share one at
            # the denominator column and corrupt it)
            pc = ps_c.tile([P, 2 * 66], F32, tag="psw", name="pc")
            # the two heads' accumulation groups must run back-to-back, not
            # interleaved: start=True clears has_written bank-wide, so an
            # interleaved second group would drop the first group's jt=0.
            for hh in range(2):
                for jt in range(ST):
                    eo = (qt // 4) * 1024 + hh * 512 + (qt % 4) * P
                    nc.tensor.matmul(
                        pc[:, hh * 66: hh * 66 + HD1],
                        lhsT=Et[jt][:, eo: eo + P],
                        rhs=Vp[:, jt * VW + hh * HD1: jt * VW + (hh + 1) * HD1],
                        start=(jt == 0), stop=(jt == ST - 1))
            rinv = sm_pool.tile([P, 2], F32, tag="rinv", bufs=4, name="rinv")
            nc.vector.reciprocal(
                rinv.rearrange("p (a b) -> p a b", b=1),
                pc.rearrange("p (g c) -> p g c", c=66)[:, :, D:D + 1])
            for hh in range(2):
                nc.vector.tensor_scalar_mul(
                    prev["out_p"][:, qt * P + hh * D: qt * P + hh * D + D],
                    pc[:, hh * 66: hh * 66 + D], rinv[:, hh:hh + 1])

        def out_dma(prev):
            nc.sync.dma_start(
                out_ext.rearrange("(t p) (g c) -> p t g c", p=P, c=P)[
                    :, :, prev["mt"], :],
                prev["out_p"].rearrange("p (t c) -> p t c", c=P))

        def pair_tiles(mt):
            QT = pp_pool.tile([P, S], BF16, tag="qt", bufs=3, name="QT")
            KT2 = pp_pool.tile([P, S], BF16, tag="kt", bufs=2, name="KT")
            Vp = pp_pool.tile([P, ST * VW], BF16, tag="vp", bufs=3, name="Vp")
            out_p = pp_pool.tile([P, ST * P], F32, tag="outp", bufs=2,
                                 name="out_p")
            return {"mt": mt, "QT": QT, "KT2": KT2, "Vp": Vp,
                    "out_p": out_p, "Et": {}}

        # ---- pair structs (allocation order = usage order) ----
        pairs = [pair_tiles(m) for m in range(NP)]

        def scores(p, sl):
            scores_ih(pairs[p]["QT"], pairs[p]["KT2"], sl // 2, sl % 2,
                      pairs[p]["Et"])

        def proj(p, nm, ih):
            proj_half(pairs[p]["QT" if nm == "q" else "KT2"], nm, p, ih)

        def vproj(p, jt):
            vproj_jt(pairs[p], jt)

        # ---- pair 0: ramp — all ih=0 slots first so the exp chain can
        # start on just the h0 data; h1 arrives while it runs ----
        def scores0(jt, ih):
            scores_ih(pairs[0]["QT"], pairs[0]["KT2"], jt, ih,
                      pairs[0]["Et"])
        proj(0, "q", 0)
        issue_gated_loads(pairs[0]["QT"])
        proj(0, "k", 0)
        scores0(0, 0)
        build_bb()
        scores0(1, 0)
        scores0(2, 0)
        vproj(0, 0)
        scores0(3, 0)
        proj(0, "k", 1)
        vproj(0, 1)
        scores0(4, 0)
        vproj(0, 2)
        scores0(5, 0)
        proj(0, "q", 1)
        scores0(6, 0)
        vproj(0, 3)
        scores0(7, 0)
        scores0(0, 1)
        vproj(0, 4)
        scores0(1, 1)
        vproj(0, 5)
        scores0(2, 1)
        vproj(0, 6)
        scores0(3, 1)
        vproj(0, 7)
        vprime_ones(pairs[0]["Vp"])
        scores0(4, 1)
        proj(1, "q", 0)
        scores0(5, 1)
        proj(1, "q", 1)
        scores0(6, 1)
        proj(1, "k", 0)
        scores0(7, 1)
        proj(1, "k", 1)
        vproj(1, 0)
        vproj(1, 1)
        vproj(1, 2)
        proj(2, "q", 0)

        # ---- steady pairs: 16 scores/exp slots per pair; fillers are
        # whole chains (contiguity rule above), sized so cumulative PE
        # work stays ahead of the exp chain; the 3-deep scores-psum
        # rotation gives two slots of slack. Slots right after the pair
        # boundary carry only exp-independent work (next-pair projections
        # and V-projections). ----
        for p in range(1, NP):
            nxt = p + 1 if p < NP - 1 else None
            nx2 = p + 2 if p < NP - 2 else None
            fillers = [
                # hill-climbed slot order (sim_sched.py): cadence 17.6us;
                # on the last pair the missing next-pair work is replaced
                # by warm-up pads so the drain doesn't run at cold clock
                lambda: vproj(p, 3),
                lambda: proj(nxt, "q", 1) if nxt is not None else None,
                lambda: None,
                lambda: (vproj(p, 4), ctx_qt(pairs[p - 1], 0), vproj(p, 5)),
                lambda: ctx_qt(pairs[p - 1], 1),
                lambda: proj(nxt, "k", 0) if nxt is not None else None,
                lambda: (ctx_qt(pairs[p - 1], 2), vproj(p, 6)),
                lambda: (ctx_qt(pairs[p - 1], 3), vproj(p, 7),
                         vprime_ones(pairs[p]["Vp"])),
                lambda: ctx_qt(pairs[p - 1], 4),
                lambda: ctx_qt(pairs[p - 1], 5),
                lambda: proj(nxt, "k", 1) if nxt is not None else None,
                lambda: ctx_qt(pairs[p - 1], 6),
                lambda: (ctx_qt(pairs[p - 1], 7), out_dma(pairs[p - 1])),
                lambda: (vproj(nxt, 0), vproj(nxt, 1))
                if nxt is not None else None,
                lambda: proj(nx2, "q", 0) if nx2 is not None else None,
                lambda: vproj(nxt, 2) if nxt is not None else None,
            ]
            for sl in range(16):
                scores(p, sl)
                fillers[sl]()

        # ---- drain: back(7), output store split so the first half
        # streams while the second half computes ----
        for qt in range(4):
            ctx_qt(pairs[NP - 1], qt)
        nc.sync.dma_start(
            out_ext.rearrange("(t p) (g c) -> p t g c", p=P, c=P)[
                :, 0:4, NP - 1, :],
            pairs[NP - 1]["out_p"].rearrange("p (t c) -> p t c", c=P)[:, 0:4])
        for qt in range(4, ST):
            ctx_qt(pairs[NP - 1], qt)
        nc.sync.dma_start(
            out_ext.rearrange("(t p) (g c) -> p t g c", p=P, c=P)[
                :, 4:8, NP - 1, :],
            pairs[NP - 1]["out_p"].rearrange("p (t c) -> p t c", c=P)[:, 4:8])

    nc.compile()
    return nc


def run(inputs, trace=False, trace_kwargs=None):
    """inputs: dict of full-shape np arrays as in reference.setup_inputs()."""
    nc = build_kernel()
    bf = ml_dtypes.bfloat16

    def tile_x(x2d):
        # [S, W] f32 -> bf16 [128, (ih, g, k2, s)]
        a = np.asarray(x2d, dtype=np.float32).T.astype(bf)      # [W, S]
        a = a.reshape(2, 4, P, 2, 512)                          # g k2 p ih s
        return np.ascontiguousarray(
            a.transpose(2, 3, 0, 1, 4).reshape(P, KT_ * S))

    def tile_w(w2d):
        a = np.asarray(w2d, dtype=np.float32).astype(bf).reshape(KT_, P, W)
        fine = np.ascontiguousarray(
            a[:, :, :FINE].transpose(1, 0, 2).reshape(P, KT_ * FINE))
        bulk = np.ascontiguousarray(
            a[:, :, FINE:].transpose(1, 0, 2).reshape(P, KT_ * (W - FINE)))
        return fine, bulk

    wqf, wqb = tile_w(inputs["Wq"])
    wkf, wkb = tile_w(inputs["Wk"])
    wvf, wvb = tile_w(inputs["Wv"])
    bq = np.asarray(inputs["bq"], dtype=np.float32)
    bk = np.asarray(inputs["bk"], dtype=np.float32)
    bv = np.asarray(inputs["bv"], dtype=np.float32)
    xf = np.asarray(inputs["from_tensor"], dtype=np.float32)
    xt = np.asarray(inputs["to_tensor"], dtype=np.float32)
    in_maps = []
    for b in range(N_CORES):
        in_maps.append({
            "xfT": tile_x(xf[b]),
            "xtT": tile_x(xt[b]),
            "Wqf": wqf, "Wqb": wqb, "bq": bq,
            "Wkf": wkf, "Wkb": wkb, "bk": bk,
            "Wvf": wvf, "Wvb": wvb, "bv": bv,
        })
    res = run_bass_kernel_spmd(nc, in_maps, core_ids=list(range(N_CORES)),
                               trace=trace, **(trace_kwargs or {}))
    out = np.stack([np.asarray(res.results[b]["out"]) for b in range(N_CORES)],
                   axis=0).astype(np.float32)
    return out, res


def kernel(**inputs):
    out, _ = run(inputs, trace=False)
    return out
